# revision 1
# baseline (speedup 1.0000x reference)
"""GravityField Trainium2 kernel.

out = U * sqrt(1 + clip(0.1 * grav, -0.9, 5) + 1e-6)
where grav[t] = phi[t] . sum_t'(phi[t'] * mass[t']), phi = sqrt(2/R)*cos(coords@W+b),
mass = softplus(relu(coords@w1+b1)@w2+b2).

Sharding: pure data-parallel over B (8 batches -> 8 cores, no communication).
Each core processes coords [8192, 64] and U [8192, 512] (= 64*8 flattened).

Structure (all fp32): pass 1 computes phiT [R, T] (range-reduced Sin; the
Scalar Engine Sin only accepts [-pi, pi]) and massT [1, T]; phi_sum comes
from a PE ones-broadcast of mass + DVE multiply/reduce against phiT (avoids
per-128-chunk transposes and N=1 matmuls, which dominate fp32 PE time);
pass 2 computes grav in [1, 512] orientation and tiny K=1 transposes back
to per-partition scale columns.
"""

import sys

sys.path.insert(0, "/opt/trn_rl_repo")

import numpy as np
from contextlib import ExitStack

import concourse.bass as bass
import concourse.bacc as bacc
import concourse.mybir as mybir
from concourse import tile
from concourse.bass_utils import run_bass_kernel_spmd
from concourse.masks import make_identity

F32 = mybir.dt.float32
AF = mybir.ActivationFunctionType
ALU = mybir.AluOpType

B, T, D, R_LR, N_RFF = 8, 8192, 64, 8, 64
F = D * R_LR  # 512 floats of U per (b, t)
STRENGTH = 0.1
HALF_PI = 1.5707963267948966
TWO_PI = 6.283185307179586
INV_2PI = 0.15915494309189535
MAGIC = 12582912.0  # 1.5 * 2**23: fp32 add/sub rounds to nearest integer
PI_CLAMP = 3.14159  # strictly inside [-pi, pi] for the ACT Sin table
BIGC = 512
N_BIG = T // BIGC  # 16
CHUNK = 128
N_CHUNKS = T // CHUNK  # 64
PHI_SUM_SCALE = STRENGTH * 2.0 / N_RFF


def build_program():
    nc = bacc.Bacc("TRN2", target_bir_lowering=False, debug=False, num_devices=8)

    u_d = nc.dram_tensor("U", [T, F], F32, kind="ExternalInput")
    coords_d = nc.dram_tensor("coords", [T, D], F32, kind="ExternalInput")
    w1_d = nc.dram_tensor("mass_w1", [D, D], F32, kind="ExternalInput")
    b1_d = nc.dram_tensor("mass_b1", [D], F32, kind="ExternalInput")
    w2_d = nc.dram_tensor("mass_w2", [D, 1], F32, kind="ExternalInput")
    b2_d = nc.dram_tensor("mass_b2", [1], F32, kind="ExternalInput")
    rffw_d = nc.dram_tensor("rff_W", [D, N_RFF], F32, kind="ExternalInput")
    rffb_d = nc.dram_tensor("rff_b", [N_RFF], F32, kind="ExternalInput")
    out_d = nc.dram_tensor("out", [T, F], F32, kind="ExternalOutput")
    mscr_d = nc.dram_tensor("mscr", [N_BIG, BIGC], F32)  # mass broadcast bounce

    with tile.TileContext(nc) as tc, ExitStack() as ctx:
        const = ctx.enter_context(tc.tile_pool(name="const", bufs=1))

        identity = const.tile([128, 128], F32)
        make_identity(nc, identity[:])

        # stationary operands must have one producing engine (PE LW micro-op
        # encodes a single semaphore wait) -> bounce DMA'd weights off DVE
        w_stage = const.tile([65, 128], F32)
        nc.sync.dma_start(w_stage[0:64, 0:64], w1_d[:, :])
        nc.sync.dma_start(w_stage[64:65, 0:64], b1_d[None, :])
        nc.sync.dma_start(w_stage[0:64, 64:128], rffw_d[:, :])
        nc.sync.dma_start(w_stage[64:65, 64:128], rffb_d[None, :])
        nc.vector.tensor_scalar_add(w_stage[64:65, 64:128], w_stage[64:65, 64:128], HALF_PI)
        w_comb = const.tile([65, 128], F32)
        nc.vector.tensor_copy(w_comb[:], w_stage[:])

        w2_stage = const.tile([D, 1], F32)
        nc.sync.dma_start(w2_stage[:], w2_d[:, :])
        w2_sb = const.tile([D, 1], F32)
        nc.vector.tensor_copy(w2_sb[:], w2_stage[:])

        one11 = const.tile([1, 1], F32)
        nc.vector.memset(one11[:], 1.0)
        ones1_64 = const.tile([1, N_RFF], F32)
        nc.vector.memset(ones1_64[:], 1.0)
        b2_sb = const.tile([1, 1], F32)
        nc.sync.dma_start(b2_sb[:], b2_d[None, :])
        b2_neg_sb = const.tile([1, 1], F32)
        nc.vector.tensor_scalar_mul(b2_neg_sb[:], b2_sb[:], -1.0)
        sqrt_bias = const.tile([128, 1], F32)
        nc.vector.memset(sqrt_bias[:], 1.000001)
        phi_sum = const.tile([N_RFF, 1], F32)

        phiT_all = const.tile([N_RFF, T], F32)   # cos features, [R, T]
        massT_all = const.tile([1, T], F32)      # -mass pre-act then -mass, [1, T]
        partials = const.tile([N_RFF, N_BIG], F32)

        coords_pool = ctx.enter_context(tc.tile_pool(name="coords", bufs=3))
        caug_pool = ctx.enter_context(tc.tile_pool(name="caug", bufs=2))
        hT_pool = ctx.enter_context(tc.tile_pool(name="hT", bufs=2))
        rr_pool = ctx.enter_context(tc.tile_pool(name="rr", bufs=2))
        phw_pool = ctx.enter_context(tc.tile_pool(name="phw", bufs=2))
        bc_pool = ctx.enter_context(tc.tile_pool(name="bc", bufs=2))
        u_pool = ctx.enter_context(tc.tile_pool(name="u", bufs=55))
        scale_pool = ctx.enter_context(tc.tile_pool(name="scale", bufs=4))

        u_tiles = []

        with (
            tc.tile_pool(name="ptr", bufs=2, space=bass.MemorySpace.PSUM) as ptr_pool,
            tc.tile_pool(name="pbig", bufs=2, space=bass.MemorySpace.PSUM) as pbig_pool,
            tc.tile_pool(name="pmT", bufs=2, space=bass.MemorySpace.PSUM) as pmT_pool,
        ):
            for c in range(N_BIG):
                tsl = slice(c * BIGC, (c + 1) * BIGC)

                ct = coords_pool.tile([128, 4 * D], F32, tag="ct")
                src = coords_d[tsl, :].rearrange("(j p) d -> p j d", p=128)
                nc.gpsimd.dma_start(ct[:].rearrange("p (j d) -> p j d", j=4), src)

                tp = ptr_pool.tile([D, BIGC], F32, tag="tp")
                for j in range(4):
                    nc.tensor.transpose(
                        tp[:, j * 128 : (j + 1) * 128],
                        ct[:, j * D : (j + 1) * D],
                        identity[:],
                    )
                caug = caug_pool.tile([D + 1, BIGC], F32, tag="caug")
                nc.vector.tensor_copy(caug[0:D, :], tp[:])
                nc.vector.memset(caug[D : D + 1, :], 1.0)

                big = pbig_pool.tile([128, BIGC], F32, tag="big")
                nc.tensor.matmul(big[:], w_comb[:], caug[:], start=True, stop=True)

                hT = hT_pool.tile([D, BIGC], F32, tag="hT")
                nc.vector.tensor_scalar_max(hT[:], big[0:D, :], 0.0)  # relu

                # mass pre-act in [1, 512] orientation: trivial weight load
                mT = pmT_pool.tile([1, BIGC], F32, tag="mT")
                nc.tensor.matmul(mT[:], w2_sb[:], hT[:], start=True, stop=True)
                # -mass = ln(sigmoid(-(pre + b2)))
                nc.scalar.activation(
                    massT_all[:, tsl], mT[:], AF.Sigmoid, bias=b2_neg_sb[:], scale=-1.0
                )
                nc.scalar.activation(massT_all[:, tsl], massT_all[:, tsl], AF.Ln)
                # phi_sum partial: broadcast mass to [R, 512] via a 0-stride
                # DRAM re-read (off-PE), then DVE mul + reduce
                nc.sync.dma_start(mscr_d[c : c + 1, :], massT_all[:, tsl])
                bc = bc_pool.tile([N_RFF, BIGC], F32, tag="bc")
                nc.sync.dma_start(bc[:], mscr_d[c : c + 1, :].to_broadcast((N_RFF, BIGC)))

                # range-reduce x -> [-pi, pi]: y = x - 2pi*round(x/2pi)
                x = big[D : 2 * D, :]
                tmp = rr_pool.tile([D, BIGC], F32, tag="tmp")
                nc.vector.tensor_scalar(
                    tmp[:], x, INV_2PI, MAGIC, op0=ALU.mult, op1=ALU.add
                )
                nc.vector.tensor_scalar(
                    tmp[:], tmp[:], MAGIC, -TWO_PI, op0=ALU.subtract, op1=ALU.mult
                )
                nc.vector.tensor_tensor(tmp[:], x, tmp[:], op=ALU.add)
                nc.vector.tensor_scalar(
                    tmp[:], tmp[:], PI_CLAMP, -PI_CLAMP, op0=ALU.min, op1=ALU.max
                )
                nc.scalar.activation(phiT_all[:, tsl], tmp[:], AF.Sin)

                phw = phw_pool.tile([N_RFF, BIGC], F32, tag="phw")
                nc.vector.tensor_tensor(phw[:], phiT_all[:, tsl], bc[:], op=ALU.mult)
                nc.vector.reduce_sum(partials[:, c : c + 1], phw[:], axis=mybir.AxisListType.X)

                for j in range(4):
                    usl = slice(c * BIGC + j * 128, c * BIGC + (j + 1) * 128)
                    ut = u_pool.tile([CHUNK, F], F32, tag="u")
                    nc.sync.dma_start(ut[:], u_d[usl, :])
                    u_tiles.append(ut)

            acc_raw = const.tile([N_RFF, 1], F32)
            nc.vector.reduce_sum(acc_raw[:], partials[:], axis=mybir.AxisListType.X)
            # massT holds -mass -> negate the fold-in scale
            nc.scalar.mul(phi_sum[:], acc_raw[:], -PHI_SUM_SCALE)

        with (
            tc.tile_pool(name="pgT", bufs=2, space=bass.MemorySpace.PSUM) as pgT_pool,
            tc.tile_pool(name="pg4", bufs=2, space=bass.MemorySpace.PSUM) as pg4_pool,
        ):
            for g in range(N_BIG):
                tsl = slice(g * BIGC, (g + 1) * BIGC)
                gT = pgT_pool.tile([1, BIGC], F32, tag="gT")
                # influence in [1, 512] orientation (scales folded into phi_sum)
                nc.tensor.matmul(gT[:], phi_sum[:], phiT_all[:, tsl], start=True, stop=True)
                gsb = scale_pool.tile([1, BIGC], F32, tag="gsb")
                nc.vector.tensor_scalar(
                    gsb[:], gT[:], -0.9, 5.0, op0=ALU.max, op1=ALU.min
                )
                pg4 = pg4_pool.tile([128, 4], F32, tag="pg4")
                for j in range(4):
                    # K=1 matmul = transpose [1,128] -> [128,1]
                    nc.tensor.matmul(
                        pg4[:, j : j + 1],
                        gsb[0:1, j * 128 : (j + 1) * 128],
                        one11[:],
                        start=True,
                        stop=True,
                    )
                sc4 = scale_pool.tile([128, 4], F32, tag="sc4")
                nc.scalar.activation(sc4[:], pg4[:], AF.Sqrt, bias=sqrt_bias[:])

                for j in range(4):
                    c = 4 * g + j
                    tslU = slice(c * CHUNK, (c + 1) * CHUNK)
                    ut = u_tiles[c]
                    if c % 2 == 0:
                        nc.vector.tensor_scalar_mul(ut[:], ut[:], sc4[:, j : j + 1])
                    else:
                        nc.scalar.mul(ut[:], ut[:], sc4[:, j : j + 1])
                    nc.sync.dma_start(out_d[tslU, :], ut[:])

    nc.compile()
    return nc


_NC_CACHE = None


def _get_program():
    global _NC_CACHE
    if _NC_CACHE is None:
        _NC_CACHE = build_program()
    return _NC_CACHE


def run(inputs: dict, trace: bool = False, tmpdir=None):
    nc = _get_program()
    U = np.ascontiguousarray(np.asarray(inputs["U"], dtype=np.float32)).reshape(B, T, F)
    coords = np.ascontiguousarray(np.asarray(inputs["coords"], dtype=np.float32))
    shared = {
        "mass_w1": np.ascontiguousarray(np.asarray(inputs["mass_w1"], np.float32)),
        "mass_b1": np.ascontiguousarray(np.asarray(inputs["mass_b1"], np.float32)),
        "mass_w2": np.ascontiguousarray(np.asarray(inputs["mass_w2"], np.float32)),
        "mass_b2": np.ascontiguousarray(np.asarray(inputs["mass_b2"], np.float32)),
        "rff_W": np.ascontiguousarray(np.asarray(inputs["rff_W"], np.float32)),
        "rff_b": np.ascontiguousarray(np.asarray(inputs["rff_b"], np.float32)),
    }
    in_maps = [{"U": U[i], "coords": coords[i], **shared} for i in range(B)]
    res = run_bass_kernel_spmd(nc, in_maps, list(range(B)), trace=trace, tmpdir=tmpdir)
    out = np.stack([res.results[i]["out"].reshape(T, D, R_LR) for i in range(B)])
    return out.astype(np.float32), res


def kernel(**inputs) -> np.ndarray:
    out, _ = run(inputs, trace=False)
    return out



# revision 8
# speedup vs baseline: 1.1416x; 1.1416x over previous
"""GravityField Trainium2 kernel.

out = U * sqrt(1 + clip(0.1 * grav, -0.9, 5) + 1e-6)
where grav[t] = phi[t] . sum_t'(phi[t'] * mass[t']), phi = sqrt(2/R)*cos(coords@W+b),
mass = softplus(relu(coords@w1+b1)@w2+b2).

Sharding: pure data-parallel over B (8 batches -> 8 cores, no communication).
Each core processes coords [8192, 64] and U [8192, 512] (= 64*8 flattened).

v2 design (fp16 PE, single-pass):
- All matmuls in fp16 (fp32 PE runs LOW_HIGH double-pass at 4 cyc/col; fp16 is
  1 cyc/col). Angle math folds 1/2pi into the RFF weights so the matmul yields
  "turns"; range reduction is one fused DVE op ((z+0.5) mod 1), and Sin gets
  scale=6.28318/bias=-3.14159 so its argument lands exactly in table range.
- Activation-table churn eliminated: Relu+Sin share the trig table set; the
  Sigmoid+Ln mass activation runs once, batched over [16, 512]; one Sqrt swap
  in phase 2. (4 table loads vs 48 in v1.)
- phi_sum: -mass bounced via DRAM f16 and broadcast-read per chunk; the
  product+reduce is one fused scalar_tensor_tensor with accum_out.
- Phase 2 computes scale columns directly as [128,1] matmuls with phiT blocks
  stationary (no K=1 transposes). t-rows are interleaved "(p j)" so coords
  DMAs are 1KB/partition and U DMAs 8KB/partition contiguous.
- U: first N_PRELOAD chunks stream into SBUF during phase 1 (read bandwidth is
  otherwise idle there); the rest stream in during phase 2 on the gpsimd queue
  overlapping the output writes on the sync queue.
"""

import sys

sys.path.insert(0, "/opt/trn_rl_repo")

import numpy as np
from contextlib import ExitStack

import concourse.bass as bass
import concourse.bacc as bacc
import concourse.mybir as mybir
from concourse import tile
from concourse.bass_utils import run_bass_kernel_spmd
from concourse.masks import make_identity

F32 = mybir.dt.float32
F16 = mybir.dt.float16
AF = mybir.ActivationFunctionType
ALU = mybir.AluOpType

B, T, D, R_LR, N_RFF = 8, 8192, 64, 8, 64
F = D * R_LR  # 512 floats of U per (b, t)
STRENGTH = 0.1
HALF_PI = 1.5707963267948966
INV_2PI = 0.15915494309189535
SIN_SCALE = 6.28318  # slightly under 2*pi: |SIN_SCALE * f| <= 3.14159 for f in [-.5,.5]
MAGIC = 12582912.0  # 1.5 * 2**23: fp32 add rounds to nearest integer
PHI_SUM_SCALE = STRENGTH * 2.0 / N_RFF
BIGC = 512
N_BIG = T // BIGC  # 16
N_PRELOAD = 8  # U chunks DMA'd during phase 1; rest streamed in phase 2


def build_program():
    nc = bacc.Bacc("TRN2", target_bir_lowering=False, debug=False, num_devices=8)

    u_d = nc.dram_tensor("U", [T, F], F32, kind="ExternalInput")
    coords_d = nc.dram_tensor("coords", [T, D], F32, kind="ExternalInput")
    w1_d = nc.dram_tensor("mass_w1", [D, D], F32, kind="ExternalInput")
    b1_d = nc.dram_tensor("mass_b1", [D], F32, kind="ExternalInput")
    w2_d = nc.dram_tensor("mass_w2", [D, 1], F32, kind="ExternalInput")
    b2_d = nc.dram_tensor("mass_b2", [1], F32, kind="ExternalInput")
    rffw_d = nc.dram_tensor("rff_W", [D, N_RFF], F32, kind="ExternalInput")
    rffb_d = nc.dram_tensor("rff_b", [N_RFF], F32, kind="ExternalInput")
    out_d = nc.dram_tensor("out", [T, F], F32, kind="ExternalOutput")
    mscr_d = nc.dram_tensor("mscr", [N_BIG, BIGC], F16)  # -mass bounce (f16)

    with tile.TileContext(nc) as tc, ExitStack() as ctx:
        const = ctx.enter_context(tc.tile_pool(name="const", bufs=1))

        identity = const.tile([128, 128], F16)
        make_identity(nc, identity[:])

        # stationary operands need one producing engine -> bounce weights off DVE
        w_stage = const.tile([65, 128], F32)
        nc.sync.dma_start(w_stage[0:64, 0:64], w1_d[:, :])
        nc.sync.dma_start(w_stage[64:65, 0:64], b1_d[None, :])
        nc.sync.dma_start(w_stage[0:64, 64:128], rffw_d[:, :])
        nc.sync.dma_start(w_stage[64:65, 64:128], rffb_d[None, :])
        # angle in turns: z = coords @ (W/2pi) + (b + pi/2)/2pi; phi = sin(2pi z)
        nc.vector.tensor_scalar_mul(w_stage[0:64, 64:128], w_stage[0:64, 64:128], INV_2PI)
        nc.vector.tensor_scalar(
            w_stage[64:65, 64:128], w_stage[64:65, 64:128], HALF_PI, INV_2PI,
            op0=ALU.add, op1=ALU.mult,
        )
        w_comb = const.tile([65, 128], F16)
        nc.vector.tensor_copy(w_comb[:], w_stage[:])

        w2_stage = const.tile([D, 1], F32)
        nc.sync.dma_start(w2_stage[:], w2_d[:, :])
        w2_sb = const.tile([D, 1], F16)
        nc.vector.tensor_copy(w2_sb[:], w2_stage[:])

        b2_stage = const.tile([N_BIG, 1], F32)
        nc.sync.dma_start(b2_stage[:], b2_d[None, :].to_broadcast((N_BIG, 1)))
        b2_neg = const.tile([N_BIG, 1], F32)
        nc.vector.tensor_scalar_mul(b2_neg[:], b2_stage[:], -1.0)

        sqrt_bias = const.tile([128, 1], F32)
        nc.vector.memset(sqrt_bias[:], 1.000001)
        zero_bias = const.tile([N_RFF, 1], F32)
        nc.vector.memset(zero_bias[:], 0.0)

        massT = const.tile([N_BIG, BIGC], F32)   # mass pre-act, row per chunk
        phiT_all = const.tile([N_RFF, T], F16)   # cos features, [R, T]
        partials = const.tile([N_RFF, N_BIG], F32)
        phi_sum16 = const.tile([N_RFF, 1], F16)

        u_tiles = [
            const.tile([128, 4 * F], F32, name=f"u{i}") for i in range(N_BIG)
        ]

        ct_pool = ctx.enter_context(tc.tile_pool(name="ct", bufs=3))
        rr_pool = ctx.enter_context(tc.tile_pool(name="rr", bufs=2))
        caug_pool = ctx.enter_context(tc.tile_pool(name="caug", bufs=2))
        # pre-set the ones row (bias input) on both rotating caug buffers
        for i in range(2):
            caug_init = caug_pool.tile([D + 1, BIGC], F16, tag="caug", name=f"ci{i}")
            nc.vector.memset(caug_init[D : D + 1, :], 1.0)
        h_pool = ctx.enter_context(tc.tile_pool(name="hT", bufs=2))
        f_pool = ctx.enter_context(tc.tile_pool(name="fm", bufs=2))
        mbc_pool = ctx.enter_context(tc.tile_pool(name="mbc", bufs=3))
        prod_pool = ctx.enter_context(tc.tile_pool(name="prod", bufs=2))
        sc_pool = ctx.enter_context(tc.tile_pool(name="sc", bufs=4))

        with (
            tc.tile_pool(name="ptp", bufs=2, space=bass.MemorySpace.PSUM) as tp_pool,
            tc.tile_pool(name="pbig", bufs=2, space=bass.MemorySpace.PSUM) as big_pool,
            tc.tile_pool(name="pmT", bufs=2, space=bass.MemorySpace.PSUM) as mT_pool,
        ):
            for c in range(N_BIG):
                tsl = slice(c * BIGC, (c + 1) * BIGC)
                # coords chunk, partition p <- rows 4p..4p+3 (1KB/partition)
                ct = ct_pool.tile([128, 4 * D], F32, tag="ct")
                nc.gpsimd.dma_start(
                    ct[:], coords_d[tsl, :].rearrange("(p j) d -> p (j d)", p=128)
                )
                if c < N_PRELOAD:
                    nc.sync.dma_start(
                        u_tiles[c][:],
                        u_d[tsl, :].rearrange("(p j) f -> p (j f)", p=128),
                    )
                ct16 = ct_pool.tile([128, 4 * D], F16, tag="ct16")
                nc.vector.tensor_copy(ct16[:], ct[:])

                # transpose 4 blocks: phiT col j*128+q <-> t-row 4q+j
                tp = tp_pool.tile([D, BIGC], F16, tag="tp")
                for j in range(4):
                    nc.tensor.transpose(
                        tp[:, j * 128 : (j + 1) * 128],
                        ct16[:, j * D : (j + 1) * D],
                        identity[:],
                    )
                caug = caug_pool.tile([D + 1, BIGC], F16, tag="caug")
                nc.vector.tensor_copy(caug[0:D, :], tp[:])

                big = big_pool.tile([128, BIGC], F32, tag="big")
                nc.tensor.matmul(big[:], w_comb[:], caug[:], start=True, stop=True)

                # mass path: relu (trig table set; no swap vs Sin)
                hT = h_pool.tile([D, BIGC], F16, tag="hT")
                nc.scalar.activation(hT[:], big[0:D, :], AF.Relu, bias=zero_bias[:])
                mT = mT_pool.tile([1, BIGC], F32, tag="mT")
                nc.tensor.matmul(mT[:], w2_sb[:], hT[:], start=True, stop=True)
                # engines can't write partition offset c (32-alignment rule);
                # copy to a partition-0 row then SBUF->SBUF DMA into massT[c]
                mrow = ct_pool.tile([1, BIGC], F32, tag="mrow")
                if c % 2 == 0:
                    nc.scalar.copy(mrow[:], mT[:])
                else:
                    nc.vector.tensor_copy(mrow[:], mT[:])
                nc.gpsimd.dma_start(massT[c : c + 1, :], mrow[:])

                # range reduce: f = z - round(z) in [-0.5, 0.5] (exact)
                rru = rr_pool.tile([D, BIGC], F32, tag="rru")
                nc.vector.tensor_scalar_add(rru[:], big[D : 2 * D, :], MAGIC)
                rrk = rr_pool.tile([D, BIGC], F32, tag="rrk")
                nc.scalar.activation(rrk[:], rru[:], AF.Copy, bias=-MAGIC)
                fm = f_pool.tile([D, BIGC], F16, tag="fm")
                nc.vector.tensor_tensor(fm[:], big[D : 2 * D, :], rrk[:], op=ALU.subtract)
                # phi = sin(2pi f) = sin(2pi z)
                nc.scalar.activation(
                    phiT_all[:, tsl], fm[:], AF.Sin,
                    bias=zero_bias[:], scale=SIN_SCALE,
                )

            # batched mass activation: -mass = ln(sigmoid(-(pre + b2)))
            msig = const.tile([N_BIG, BIGC], F32)
            nc.scalar.activation(msig[:], massT[:], AF.Sigmoid, bias=b2_neg[:], scale=-1.0)
            negm16 = const.tile([N_BIG, BIGC], F16)
            nc.scalar.activation(negm16[:], msig[:], AF.Ln)
            nc.sync.dma_start(mscr_d[:, :], negm16[:])

            # phi_sum partials: broadcast -mass over R partitions, fused mul+reduce
            for c in range(N_BIG):
                tsl = slice(c * BIGC, (c + 1) * BIGC)
                mbc = mbc_pool.tile([N_RFF, BIGC], F16, tag="mbc")
                nc.gpsimd.dma_start(
                    mbc[:], mscr_d[c : c + 1, :].to_broadcast((N_RFF, BIGC))
                )
                prod = prod_pool.tile([N_RFF, BIGC], F16, tag="prod")
                nc.vector.scalar_tensor_tensor(
                    prod[:], phiT_all[:, tsl], 1.0, mbc[:],
                    op0=ALU.mult, op1=ALU.mult,
                    accum_out=partials[:, c : c + 1],
                )

            # stream in the remaining U chunks (gpsimd queue, overlaps phase 2 writes)
            for c in range(N_PRELOAD, N_BIG):
                tsl = slice(c * BIGC, (c + 1) * BIGC)
                nc.gpsimd.dma_start(
                    u_tiles[c][:], u_d[tsl, :].rearrange("(p j) f -> p (j f)", p=128)
                )

            acc_raw = const.tile([N_RFF, 1], F32)
            nc.vector.reduce_sum(acc_raw[:], partials[:], axis=mybir.AxisListType.X)
            # partials hold -sum(phi*mass); fold in -PHI_SUM_SCALE and cast f16
            nc.scalar.mul(phi_sum16[:], acc_raw[:], -PHI_SUM_SCALE)

        with tc.tile_pool(name="pg", bufs=2, space=bass.MemorySpace.PSUM) as pg_pool:
            for g in range(N_BIG):
                tsl = slice(g * BIGC, (g + 1) * BIGC)
                # influence columns: pg4[q, j] = grav(t = g*512 + 4q + j) * 0.1*2/R
                pg4 = pg_pool.tile([128, 4], F32, tag="pg4")
                for j in range(4):
                    nc.tensor.matmul(
                        pg4[:, j : j + 1],
                        phiT_all[:, g * BIGC + j * 128 : g * BIGC + (j + 1) * 128],
                        phi_sum16[:],
                        start=True, stop=True,
                    )
                infl = sc_pool.tile([128, 4], F32, tag="infl")
                nc.vector.tensor_scalar(
                    infl[:], pg4[:], -0.9, 5.0, op0=ALU.max, op1=ALU.min
                )
                sc4 = sc_pool.tile([128, 4], F32, tag="sc4")
                nc.scalar.activation(sc4[:], infl[:], AF.Sqrt, bias=sqrt_bias[:])

                ut = u_tiles[g]
                for j in range(4):
                    usl = slice(j * F, (j + 1) * F)
                    if j % 2 == 0:
                        nc.vector.tensor_scalar_mul(ut[:, usl], ut[:, usl], sc4[:, j : j + 1])
                    else:
                        nc.scalar.mul(ut[:, usl], ut[:, usl], sc4[:, j : j + 1])
                nc.sync.dma_start(
                    out_d[tsl, :].rearrange("(p j) f -> p (j f)", p=128), ut[:]
                )

    nc.compile()
    return nc


_NC_CACHE = None


def _get_program():
    global _NC_CACHE
    if _NC_CACHE is None:
        _NC_CACHE = build_program()
    return _NC_CACHE


def run(inputs: dict, trace: bool = False, tmpdir=None):
    nc = _get_program()
    U = np.ascontiguousarray(np.asarray(inputs["U"], dtype=np.float32)).reshape(B, T, F)
    coords = np.ascontiguousarray(np.asarray(inputs["coords"], dtype=np.float32))
    shared = {
        "mass_w1": np.ascontiguousarray(np.asarray(inputs["mass_w1"], np.float32)),
        "mass_b1": np.ascontiguousarray(np.asarray(inputs["mass_b1"], np.float32)),
        "mass_w2": np.ascontiguousarray(np.asarray(inputs["mass_w2"], np.float32)),
        "mass_b2": np.ascontiguousarray(np.asarray(inputs["mass_b2"], np.float32)),
        "rff_W": np.ascontiguousarray(np.asarray(inputs["rff_W"], np.float32)),
        "rff_b": np.ascontiguousarray(np.asarray(inputs["rff_b"], np.float32)),
    }
    in_maps = [{"U": U[i], "coords": coords[i], **shared} for i in range(B)]
    res = run_bass_kernel_spmd(nc, in_maps, list(range(B)), trace=trace, tmpdir=tmpdir)
    out = np.stack([res.results[i]["out"].reshape(T, D, R_LR) for i in range(B)])
    return out.astype(np.float32), res


def kernel(**inputs) -> np.ndarray:
    out, _ = run(inputs, trace=False)
    return out


# revision 9
# speedup vs baseline: 1.2099x; 1.0598x over previous
"""GravityField Trainium2 kernel.

out = U * sqrt(1 + clip(0.1 * grav, -0.9, 5) + 1e-6)
where grav[t] = phi[t] . sum_t'(phi[t'] * mass[t']), phi = sqrt(2/R)*cos(coords@W+b),
mass = softplus(relu(coords@w1+b1)@w2+b2).

Sharding: pure data-parallel over B (8 batches -> 8 cores, no communication).
Each core processes coords [8192, 64] and U [8192, 512] (= 64*8 flattened).

v3 design (fp16 PE; DMA-floor ~85us at the ~420 GB/s shared R+W per-core rate):
- All matmuls fp16 (fp32 PE runs LOW_HIGH double-pass at 4 cyc/col). The RFF
  weights fold 1/2pi so the matmul yields angle-in-turns z; range reduction is
  two DVE ops: u = z + MAGIC, then fm = (u - MAGIC) - z = round(z) - z = -f via
  one scalar_tensor_tensor. phiT therefore holds -cos; the sign cancels because
  phi enters grav quadratically. Sin gets scale=6.28318 so |arg| <= 3.14159.
- Table churn: Relu+Sin co-reside in the trig act set; Sigmoid+Ln run once,
  batched over massT [16, 512]; one Sqrt set in phase 2.
- phi_sum: -mass bounced to DRAM f16, broadcast-read in 4 groups of 4 chunks,
  fused multiply+accumulate via scalar_tensor_tensor(accum_out).
- Phase 2 scale columns come straight from [128,1] matmuls with phiT blocks
  stationary (no K=1 transposes). t-rows interleave "(p j)" so coords DMAs are
  1KB/partition and U DMAs 8KB/partition contiguous.
- Queues: sync carries the 16 1MB U preloads from t=0 then the output writes;
  gpsimd carries weights, coords, massT row gathers, and the mass broadcasts.
"""

import sys

sys.path.insert(0, "/opt/trn_rl_repo")

import numpy as np
from contextlib import ExitStack

import concourse.bass as bass
import concourse.bacc as bacc
import concourse.mybir as mybir
from concourse import tile
from concourse.bass_utils import run_bass_kernel_spmd
from concourse.masks import make_identity

F32 = mybir.dt.float32
F16 = mybir.dt.float16
AF = mybir.ActivationFunctionType
ALU = mybir.AluOpType

B, T, D, R_LR, N_RFF = 8, 8192, 64, 8, 64
F = D * R_LR  # 512 floats of U per (b, t)
STRENGTH = 0.1
HALF_PI = 1.5707963267948966
INV_2PI = 0.15915494309189535
SIN_SCALE = 6.28318  # slightly under 2*pi: |SIN_SCALE * f| <= 3.14159 for f in [-.5,.5]
MAGIC = 12582912.0  # 1.5 * 2**23: fp32 add rounds to nearest integer
PHI_SUM_SCALE = STRENGTH * 2.0 / N_RFF
BIGC = 512
N_BIG = T // BIGC  # 16
PGRP = 4  # chunks per phi_sum product group


def build_program():
    nc = bacc.Bacc("TRN2", target_bir_lowering=False, debug=False, num_devices=8)

    u_d = nc.dram_tensor("U", [T, F], F32, kind="ExternalInput")
    coords_d = nc.dram_tensor("coords", [T, D], F32, kind="ExternalInput")
    w1_d = nc.dram_tensor("mass_w1", [D, D], F32, kind="ExternalInput")
    b1_d = nc.dram_tensor("mass_b1", [D], F32, kind="ExternalInput")
    w2_d = nc.dram_tensor("mass_w2", [D, 1], F32, kind="ExternalInput")
    b2_d = nc.dram_tensor("mass_b2", [1], F32, kind="ExternalInput")
    rffw_d = nc.dram_tensor("rff_W", [D, N_RFF], F32, kind="ExternalInput")
    rffb_d = nc.dram_tensor("rff_b", [N_RFF], F32, kind="ExternalInput")
    out_d = nc.dram_tensor("out", [T, F], F32, kind="ExternalOutput")
    mscr_d = nc.dram_tensor("mscr", [1, T], F16)  # -mass bounce (f16)

    with tile.TileContext(nc) as tc, ExitStack() as ctx:
        const = ctx.enter_context(tc.tile_pool(name="const", bufs=1))

        u_tiles = [
            const.tile([128, 4 * F], F32, name=f"u{i}") for i in range(N_BIG)
        ]

        identity = const.tile([128, 128], F32)
        make_identity(nc, identity[:])

        # stationary operands need one producing engine -> bounce weights off DVE
        w_stage = const.tile([65, 128], F32)
        nc.gpsimd.dma_start(w_stage[0:64, 0:64], w1_d[:, :])
        nc.gpsimd.dma_start(w_stage[64:65, 0:64], b1_d[None, :])
        nc.gpsimd.dma_start(w_stage[0:64, 64:128], rffw_d[:, :])
        nc.gpsimd.dma_start(w_stage[64:65, 64:128], rffb_d[None, :])
        # angle in turns: z = coords @ (W/2pi) + (b + pi/2)/2pi; phi = sin(2pi z)
        nc.vector.tensor_scalar_mul(w_stage[0:64, 64:128], w_stage[0:64, 64:128], INV_2PI)
        nc.vector.tensor_scalar(
            w_stage[64:65, 64:128], w_stage[64:65, 64:128], HALF_PI, INV_2PI,
            op0=ALU.add, op1=ALU.mult,
        )
        w_comb = const.tile([65, 128], F16)
        nc.vector.tensor_copy(w_comb[:], w_stage[:])

        w2_stage = const.tile([D, 1], F32)
        nc.gpsimd.dma_start(w2_stage[:], w2_d[:, :])
        w2_sb = const.tile([D, 1], F16)
        nc.vector.tensor_copy(w2_sb[:], w2_stage[:])

        b2_stage = const.tile([N_BIG, 1], F32)
        nc.gpsimd.dma_start(b2_stage[:], b2_d[None, :].to_broadcast((N_BIG, 1)))
        b2_neg = const.tile([N_BIG, 1], F32)
        nc.vector.tensor_scalar_mul(b2_neg[:], b2_stage[:], -1.0)

        sqrt_bias = const.tile([128, 1], F32)
        nc.vector.memset(sqrt_bias[:], 1.000001)
        zero_bias = const.tile([N_RFF, 1], F32)
        nc.vector.memset(zero_bias[:], 0.0)

        massT = const.tile([N_BIG, BIGC], F32)   # mass pre-act, row per chunk
        phiT_all = const.tile([N_RFF, T], F16)   # -cos features, [R, T]
        partials = const.tile([N_RFF, N_BIG // PGRP], F32)
        phi_sum16 = const.tile([N_RFF, 1], F16)

        ct_pool = ctx.enter_context(tc.tile_pool(name="ct", bufs=4))
        rr_pool = ctx.enter_context(tc.tile_pool(name="rr", bufs=3))
        caug_pool = ctx.enter_context(tc.tile_pool(name="caug", bufs=3))
        # pre-set the ones row (bias input) on the rotating caug buffers
        for i in range(3):
            caug_init = caug_pool.tile([D + 1, BIGC], F16, tag="caug", name=f"ci{i}")
            nc.vector.memset(caug_init[D : D + 1, :], 1.0)
        h_pool = ctx.enter_context(tc.tile_pool(name="hT", bufs=2))
        f_pool = ctx.enter_context(tc.tile_pool(name="fm", bufs=3))
        mbc_pool = ctx.enter_context(tc.tile_pool(name="mbc", bufs=2))
        prod_pool = ctx.enter_context(tc.tile_pool(name="prod", bufs=2))
        sc_pool = ctx.enter_context(tc.tile_pool(name="sc", bufs=4))

        with (
            tc.tile_pool(name="ptp", bufs=2, space=bass.MemorySpace.PSUM) as tp_pool,
            tc.tile_pool(name="pbig", bufs=3, space=bass.MemorySpace.PSUM) as big_pool,
            tc.tile_pool(name="pmT", bufs=2, space=bass.MemorySpace.PSUM) as mT_pool,
        ):
            for c in range(N_BIG):
                tsl = slice(c * BIGC, (c + 1) * BIGC)
                nc.sync.dma_start(
                    u_tiles[c][:],
                    u_d[tsl, :].rearrange("(p j) f -> p (j f)", p=128),
                )
                # coords chunk, partition p <- rows 4p..4p+3 (1KB/partition)
                ct = ct_pool.tile([128, 4 * D], F32, tag="ct")
                nc.gpsimd.dma_start(
                    ct[:], coords_d[tsl, :].rearrange("(p j) d -> p (j d)", p=128)
                )

                # transpose 4 blocks (fp32): phiT col j*128+q <-> t-row 4q+j
                tp = tp_pool.tile([D, BIGC], F32, tag="tp")
                for j in range(4):
                    nc.tensor.transpose(
                        tp[:, j * 128 : (j + 1) * 128],
                        ct[:, j * D : (j + 1) * D],
                        identity[:],
                    )
                caug = caug_pool.tile([D + 1, BIGC], F16, tag="caug")
                nc.vector.tensor_copy(caug[0:D, :], tp[:])

                big = big_pool.tile([128, BIGC], F32, tag="big")
                nc.tensor.matmul(big[:], w_comb[:], caug[:], start=True, stop=True)

                # mass path: relu (trig act set; no table swap vs Sin)
                hT = h_pool.tile([D, BIGC], F16, tag="hT")
                nc.scalar.activation(hT[:], big[0:D, :], AF.Relu, bias=zero_bias[:])
                mT = mT_pool.tile([1, BIGC], F32, tag="mT")
                nc.tensor.matmul(mT[:], w2_sb[:], hT[:], start=True, stop=True)
                # engines can't write partition offset c (32-alignment rule);
                # copy to a partition-0 row then SBUF->SBUF DMA into massT[c]
                mrow = ct_pool.tile([1, BIGC], F32, tag="mrow", bufs=2)
                if c % 2 == 0:
                    nc.scalar.copy(mrow[:], mT[:])
                else:
                    nc.vector.tensor_copy(mrow[:], mT[:])
                nc.gpsimd.dma_start(massT[c : c + 1, :], mrow[:])

                # range reduce: fm = round(z) - z = -f, exact, |fm| <= 0.5
                rru = rr_pool.tile([D, BIGC], F32, tag="rru")
                nc.vector.tensor_scalar_add(rru[:], big[D : 2 * D, :], MAGIC)
                fm = f_pool.tile([D, BIGC], F16, tag="fm")
                nc.vector.scalar_tensor_tensor(
                    fm[:], rru[:], MAGIC, big[D : 2 * D, :],
                    op0=ALU.subtract, op1=ALU.subtract,
                )
                # phiT = sin(2pi * -f) = -cos(angle); sign cancels in grav
                nc.scalar.activation(
                    phiT_all[:, tsl], fm[:], AF.Sin,
                    bias=zero_bias[:], scale=SIN_SCALE,
                )

            # batched mass activation: -mass = ln(sigmoid(-(pre + b2)))
            msig = const.tile([N_BIG, BIGC], F32)
            nc.scalar.activation(msig[:], massT[:], AF.Sigmoid, bias=b2_neg[:], scale=-1.0)
            negm16 = const.tile([N_BIG, BIGC], F16)
            nc.scalar.activation(negm16[:], msig[:], AF.Ln)
            nc.gpsimd.dma_start(
                mscr_d[:, :].rearrange("a (c q) -> (a c) q", c=N_BIG), negm16[:]
            )

            # phi_sum partials: bcast -mass over R partitions, fused mul+accum
            for g in range(N_BIG // PGRP):
                gsl = slice(g * PGRP * BIGC, (g + 1) * PGRP * BIGC)
                mbc = mbc_pool.tile([N_RFF, PGRP * BIGC], F16, tag="mbc")
                nc.gpsimd.dma_start(
                    mbc[:], mscr_d[:, gsl].to_broadcast((N_RFF, PGRP * BIGC))
                )
                prod = prod_pool.tile([N_RFF, PGRP * BIGC], F16, tag="prod")
                nc.vector.scalar_tensor_tensor(
                    prod[:], phiT_all[:, gsl], 1.0, mbc[:],
                    op0=ALU.mult, op1=ALU.mult,
                    accum_out=partials[:, g : g + 1],
                )

            acc_raw = const.tile([N_RFF, 1], F32)
            nc.vector.reduce_sum(acc_raw[:], partials[:], axis=mybir.AxisListType.X)
            # acc = sum(phi*mass) (two sign flips cancel); phiT holds -cos, so
            # phi_sum16 = -PHI_SUM_SCALE * acc makes pg4 = +0.1*grav
            nc.scalar.mul(phi_sum16[:], acc_raw[:], -PHI_SUM_SCALE)

        with tc.tile_pool(name="pg", bufs=2, space=bass.MemorySpace.PSUM) as pg_pool:
            for g in range(N_BIG):
                tsl = slice(g * BIGC, (g + 1) * BIGC)
                # influence columns: pg4[q, j] = 0.1*grav(t = g*512 + 4q + j)
                pg4 = pg_pool.tile([128, 4], F32, tag="pg4")
                for j in range(4):
                    nc.tensor.matmul(
                        pg4[:, j : j + 1],
                        phiT_all[:, g * BIGC + j * 128 : g * BIGC + (j + 1) * 128],
                        phi_sum16[:],
                        start=True, stop=True,
                    )
                infl = sc_pool.tile([128, 4], F32, tag="infl")
                nc.vector.tensor_scalar(
                    infl[:], pg4[:], -0.9, 5.0, op0=ALU.max, op1=ALU.min
                )
                sc4 = sc_pool.tile([128, 4], F32, tag="sc4")
                nc.scalar.activation(sc4[:], infl[:], AF.Sqrt, bias=sqrt_bias[:])

                ut = u_tiles[g]
                for j in range(4):
                    usl = slice(j * F, (j + 1) * F)
                    if j % 2 == 0:
                        nc.vector.tensor_scalar_mul(ut[:, usl], ut[:, usl], sc4[:, j : j + 1])
                    else:
                        nc.scalar.mul(ut[:, usl], ut[:, usl], sc4[:, j : j + 1])
                nc.sync.dma_start(
                    out_d[tsl, :].rearrange("(p j) f -> p (j f)", p=128), ut[:]
                )

    nc.compile()
    return nc


_NC_CACHE = None


def _get_program():
    global _NC_CACHE
    if _NC_CACHE is None:
        _NC_CACHE = build_program()
    return _NC_CACHE


def run(inputs: dict, trace: bool = False, tmpdir=None):
    nc = _get_program()
    U = np.ascontiguousarray(np.asarray(inputs["U"], dtype=np.float32)).reshape(B, T, F)
    coords = np.ascontiguousarray(np.asarray(inputs["coords"], dtype=np.float32))
    shared = {
        "mass_w1": np.ascontiguousarray(np.asarray(inputs["mass_w1"], np.float32)),
        "mass_b1": np.ascontiguousarray(np.asarray(inputs["mass_b1"], np.float32)),
        "mass_w2": np.ascontiguousarray(np.asarray(inputs["mass_w2"], np.float32)),
        "mass_b2": np.ascontiguousarray(np.asarray(inputs["mass_b2"], np.float32)),
        "rff_W": np.ascontiguousarray(np.asarray(inputs["rff_W"], np.float32)),
        "rff_b": np.ascontiguousarray(np.asarray(inputs["rff_b"], np.float32)),
    }
    in_maps = [{"U": U[i], "coords": coords[i], **shared} for i in range(B)]
    res = run_bass_kernel_spmd(nc, in_maps, list(range(B)), trace=trace, tmpdir=tmpdir)
    out = np.stack([res.results[i]["out"].reshape(T, D, R_LR) for i in range(B)])
    return out.astype(np.float32), res


def kernel(**inputs) -> np.ndarray:
    out, _ = run(inputs, trace=False)
    return out


# revision 11
# speedup vs baseline: 1.2851x; 1.0622x over previous
"""GravityField Trainium2 kernel.

out = U * sqrt(1 + clip(0.1 * grav, -0.9, 5) + 1e-6)
where grav[t] = phi[t] . sum_t'(phi[t'] * mass[t']), phi = sqrt(2/R)*cos(coords@W+b),
mass = softplus(relu(coords@w1+b1)@w2+b2).

Sharding: pure data-parallel over B (8 batches -> 8 cores, no communication).
Each core processes coords [8192, 64] and U [8192, 512] (= 64*8 flattened).

v3 design (fp16 PE; DMA-floor ~85us at the ~420 GB/s shared R+W per-core rate):
- All matmuls fp16 (fp32 PE runs LOW_HIGH double-pass at 4 cyc/col). The RFF
  weights fold 1/2pi so the matmul yields angle-in-turns z; range reduction is
  two DVE ops: u = z + MAGIC, then fm = (u - MAGIC) - z = round(z) - z = -f via
  one scalar_tensor_tensor. phiT therefore holds -cos; the sign cancels because
  phi enters grav quadratically. Sin gets scale=6.28318 so |arg| <= 3.14159.
- Table churn: Relu+Sin co-reside in the trig act set; Sigmoid+Ln run once,
  batched over massT [16, 512]; one Sqrt set in phase 2.
- phi_sum: -mass bounced to DRAM f16, broadcast-read in 4 groups of 4 chunks,
  fused multiply+accumulate via scalar_tensor_tensor(accum_out).
- Phase 2 scale columns come straight from [128,1] matmuls with phiT blocks
  stationary (no K=1 transposes). t-rows interleave "(p j)" so coords DMAs are
  1KB/partition and U DMAs 8KB/partition contiguous.
- Queues: sync carries the 16 1MB U preloads from t=0 then the output writes;
  gpsimd carries weights, coords, massT row gathers, and the mass broadcasts.
"""

import sys

sys.path.insert(0, "/opt/trn_rl_repo")

import numpy as np
from contextlib import ExitStack

import concourse.bass as bass
import concourse.bacc as bacc
import concourse.mybir as mybir
from concourse import tile
from concourse.bass_utils import run_bass_kernel_spmd
from concourse.masks import make_identity

F32 = mybir.dt.float32
F16 = mybir.dt.float16
AF = mybir.ActivationFunctionType
ALU = mybir.AluOpType

B, T, D, R_LR, N_RFF = 8, 8192, 64, 8, 64
F = D * R_LR  # 512 floats of U per (b, t)
STRENGTH = 0.1
HALF_PI = 1.5707963267948966
INV_2PI = 0.15915494309189535
SIN_SCALE = 6.28318  # slightly under 2*pi: |SIN_SCALE * f| <= 3.14159 for f in [-.5,.5]
MAGIC = 12582912.0  # 1.5 * 2**23: fp32 add rounds to nearest integer
PHI_SUM_SCALE = STRENGTH * 2.0 / N_RFF
BIGC = 512
N_BIG = T // BIGC  # 16


def build_program():
    nc = bacc.Bacc("TRN2", target_bir_lowering=False, debug=False, num_devices=8)

    u_d = nc.dram_tensor("U", [T, F], F32, kind="ExternalInput")
    coords_d = nc.dram_tensor("coords", [T, D], F32, kind="ExternalInput")
    w1_d = nc.dram_tensor("mass_w1", [D, D], F32, kind="ExternalInput")
    b1_d = nc.dram_tensor("mass_b1", [D], F32, kind="ExternalInput")
    w2_d = nc.dram_tensor("mass_w2", [D, 1], F32, kind="ExternalInput")
    b2_d = nc.dram_tensor("mass_b2", [1], F32, kind="ExternalInput")
    rffw_d = nc.dram_tensor("rff_W", [D, N_RFF], F32, kind="ExternalInput")
    rffb_d = nc.dram_tensor("rff_b", [N_RFF], F32, kind="ExternalInput")
    out_d = nc.dram_tensor("out", [T, F], F32, kind="ExternalOutput")
    mscr_d = nc.dram_tensor("mscr", [1, T], F16)  # -mass bounce (f16)

    with tile.TileContext(nc) as tc, ExitStack() as ctx:
        const = ctx.enter_context(tc.tile_pool(name="const", bufs=1))

        u_tiles = [
            const.tile([128, 4 * F], F32, name=f"u{i}") for i in range(N_BIG)
        ]

        identity = const.tile([128, 128], F32)
        make_identity(nc, identity[:])

        # stationary operands need one producing engine -> bounce weights off DVE
        w_stage = const.tile([65, 128], F32)
        nc.scalar.dma_start(w_stage[0:64, 0:64], w1_d[:, :])
        nc.scalar.dma_start(w_stage[64:65, 0:64], b1_d[None, :])
        nc.scalar.dma_start(w_stage[0:64, 64:128], rffw_d[:, :])
        nc.scalar.dma_start(w_stage[64:65, 64:128], rffb_d[None, :])
        # angle in turns: z = coords @ (W/2pi) + (b + pi/2)/2pi; phi = sin(2pi z)
        nc.vector.tensor_scalar_mul(w_stage[0:64, 64:128], w_stage[0:64, 64:128], INV_2PI)
        nc.vector.tensor_scalar(
            w_stage[64:65, 64:128], w_stage[64:65, 64:128], HALF_PI, INV_2PI,
            op0=ALU.add, op1=ALU.mult,
        )
        w_comb = const.tile([65, 128], F16)
        nc.vector.tensor_copy(w_comb[:], w_stage[:])

        w2_stage = const.tile([D, 1], F32)
        nc.scalar.dma_start(w2_stage[:], w2_d[:, :])
        w2_sb = const.tile([D, 1], F16)
        nc.vector.tensor_copy(w2_sb[:], w2_stage[:])

        b2_stage = const.tile([4, 1], F32)
        nc.scalar.dma_start(b2_stage[:], b2_d[None, :].to_broadcast((4, 1)))
        b2_neg = const.tile([4, 1], F32)
        nc.vector.tensor_scalar_mul(b2_neg[:], b2_stage[:], -1.0)

        sqrt_bias = const.tile([128, 1], F32)
        nc.vector.memset(sqrt_bias[:], 1.000001)
        zero_bias = const.tile([N_RFF, 1], F32)
        nc.vector.memset(zero_bias[:], 0.0)

        # mass pre-act, one [4, 512] tile per quarter (partition-0-aligned so
        # Sigmoid/Ln can run per quarter, overlapping the main loop)
        massT_q = [const.tile([4, BIGC], F32, name=f"mq{i}") for i in range(4)]
        msig_q = [const.tile([4, BIGC], F32, name=f"ms{i}") for i in range(4)]
        negm_q = [const.tile([4, BIGC], F16, name=f"mn{i}") for i in range(4)]
        phiT_all = const.tile([N_RFF, T], F16)   # -cos features, [R, T]
        partials = const.tile([N_RFF, 4], F32)
        phi_sum16 = const.tile([N_RFF, 1], F16)

        ct_pool = ctx.enter_context(tc.tile_pool(name="ct", bufs=4))
        rr_pool = ctx.enter_context(tc.tile_pool(name="rr", bufs=3))
        caug_pool = ctx.enter_context(tc.tile_pool(name="caug", bufs=3))
        # pre-set the ones row (bias input) on the rotating caug buffers
        for i in range(3):
            caug_init = caug_pool.tile([D + 1, BIGC], F16, tag="caug", name=f"ci{i}")
            nc.vector.memset(caug_init[D : D + 1, :], 1.0)
        h_pool = ctx.enter_context(tc.tile_pool(name="hT", bufs=2))
        f_pool = ctx.enter_context(tc.tile_pool(name="fm", bufs=3))
        mbc_pool = ctx.enter_context(tc.tile_pool(name="mbc", bufs=3))
        prod_pool = ctx.enter_context(tc.tile_pool(name="prod", bufs=2))
        sc_pool = ctx.enter_context(tc.tile_pool(name="sc", bufs=4))

        with (
            tc.tile_pool(name="ptp", bufs=2, space=bass.MemorySpace.PSUM) as tp_pool,
            tc.tile_pool(name="pbig", bufs=4, space=bass.MemorySpace.PSUM) as big_pool,
            tc.tile_pool(name="pmT", bufs=2, space=bass.MemorySpace.PSUM) as mT_pool,
        ):
            for c in range(N_BIG):
                tsl = slice(c * BIGC, (c + 1) * BIGC)
                nc.sync.dma_start(
                    u_tiles[c][:],
                    u_d[tsl, :].rearrange("(p j) f -> p (j f)", p=128),
                )
                # coords chunk, partition p <- rows 4p..4p+3 (1KB/partition)
                ct = ct_pool.tile([128, 4 * D], F32, tag="ct")
                nc.gpsimd.dma_start(
                    ct[:], coords_d[tsl, :].rearrange("(p j) d -> p (j d)", p=128)
                )

                # transpose 4 blocks (fp32): phiT col j*128+q <-> t-row 4q+j
                tp = tp_pool.tile([D, BIGC], F32, tag="tp")
                for j in range(4):
                    nc.tensor.transpose(
                        tp[:, j * 128 : (j + 1) * 128],
                        ct[:, j * D : (j + 1) * D],
                        identity[:],
                    )
                caug = caug_pool.tile([D + 1, BIGC], F16, tag="caug")
                nc.vector.tensor_copy(caug[0:D, :], tp[:])

                big = big_pool.tile([128, BIGC], F32, tag="big")
                nc.tensor.matmul(big[:], w_comb[:], caug[:], start=True, stop=True)

                # mass path: relu (trig act set; no table swap vs Sin)
                hT = h_pool.tile([D, BIGC], F16, tag="hT")
                nc.scalar.activation(hT[:], big[0:D, :], AF.Relu, bias=zero_bias[:])
                mT = mT_pool.tile([1, BIGC], F32, tag="mT")
                nc.tensor.matmul(mT[:], w2_sb[:], hT[:], start=True, stop=True)

                # range reduce: fm = round(z) - z = -f, exact, |fm| <= 0.5
                rru = rr_pool.tile([D, BIGC], F32, tag="rru")
                nc.vector.tensor_scalar_add(rru[:], big[D : 2 * D, :], MAGIC)
                fm = f_pool.tile([D, BIGC], F16, tag="fm")
                nc.vector.scalar_tensor_tensor(
                    fm[:], rru[:], MAGIC, big[D : 2 * D, :],
                    op0=ALU.subtract, op1=ALU.subtract,
                )
                # phiT = sin(2pi * -f) = -cos(angle); sign cancels in grav
                nc.scalar.activation(
                    phiT_all[:, tsl], fm[:], AF.Sin,
                    bias=zero_bias[:], scale=SIN_SCALE,
                )
                # engines can't write partition offset c (32-alignment rule);
                # copy to a partition-0 row then SBUF->SBUF DMA into massT[c]
                mrow = ct_pool.tile([1, BIGC], F32, tag="mrow", bufs=2)
                if c % 2 == 0:
                    nc.scalar.copy(mrow[:], mT[:])
                else:
                    nc.vector.tensor_copy(mrow[:], mT[:])
                nc.gpsimd.dma_start(massT_q[c // 4][c % 4 : c % 4 + 1, :], mrow[:])

                if c % 4 == 3:
                    g = c // 4
                    gsl = slice(g * 4 * BIGC, (g + 1) * 4 * BIGC)
                    nc.scalar.activation(
                        msig_q[g][:], massT_q[g][:], AF.Sigmoid,
                        bias=b2_neg[:], scale=-1.0,
                    )
                    nc.scalar.activation(negm_q[g][:], msig_q[g][:], AF.Ln)
                    nc.gpsimd.dma_start(
                        mscr_d[:, gsl].rearrange("a (r q) -> (a r) q", r=4),
                        negm_q[g][:],
                    )
                    mbc = mbc_pool.tile([N_RFF, 4 * BIGC], F16, tag="mbc")
                    nc.gpsimd.dma_start(
                        mbc[:], mscr_d[:, gsl].to_broadcast((N_RFF, 4 * BIGC))
                    )
                    prod = prod_pool.tile([N_RFF, 4 * BIGC], F16, tag="prod")
                    nc.vector.scalar_tensor_tensor(
                        prod[:], phiT_all[:, gsl], 1.0, mbc[:],
                        op0=ALU.mult, op1=ALU.mult,
                        accum_out=partials[:, g : g + 1],
                    )

            acc_raw = const.tile([N_RFF, 1], F32)
            nc.vector.reduce_sum(acc_raw[:], partials[:], axis=mybir.AxisListType.X)
            # acc = sum(phi*mass) (two sign flips cancel); phiT holds -cos, so
            # phi_sum16 = -PHI_SUM_SCALE * acc makes pg4 = +0.1*grav
            nc.scalar.mul(phi_sum16[:], acc_raw[:], -PHI_SUM_SCALE)

        with tc.tile_pool(name="pg", bufs=2, space=bass.MemorySpace.PSUM) as pg_pool:
            for g in range(N_BIG):
                tsl = slice(g * BIGC, (g + 1) * BIGC)
                # influence columns: pg4[q, j] = 0.1*grav(t = g*512 + 4q + j)
                pg4 = pg_pool.tile([128, 4], F32, tag="pg4")
                for j in range(4):
                    nc.tensor.matmul(
                        pg4[:, j : j + 1],
                        phiT_all[:, g * BIGC + j * 128 : g * BIGC + (j + 1) * 128],
                        phi_sum16[:],
                        start=True, stop=True,
                    )
                infl = sc_pool.tile([128, 4], F32, tag="infl")
                nc.vector.tensor_scalar(
                    infl[:], pg4[:], -0.9, 5.0, op0=ALU.max, op1=ALU.min
                )
                sc4 = sc_pool.tile([128, 4], F32, tag="sc4")
                nc.scalar.activation(sc4[:], infl[:], AF.Sqrt, bias=sqrt_bias[:])

                ut = u_tiles[g]
                for j in range(4):
                    usl = slice(j * F, (j + 1) * F)
                    if j % 2 == 0:
                        nc.vector.tensor_scalar_mul(ut[:, usl], ut[:, usl], sc4[:, j : j + 1])
                    else:
                        nc.scalar.mul(ut[:, usl], ut[:, usl], sc4[:, j : j + 1])
                nc.sync.dma_start(
                    out_d[tsl, :].rearrange("(p j) f -> p (j f)", p=128), ut[:]
                )

    nc.compile()
    return nc


_NC_CACHE = None


def _get_program():
    global _NC_CACHE
    if _NC_CACHE is None:
        _NC_CACHE = build_program()
    return _NC_CACHE


def run(inputs: dict, trace: bool = False, tmpdir=None):
    nc = _get_program()
    U = np.ascontiguousarray(np.asarray(inputs["U"], dtype=np.float32)).reshape(B, T, F)
    coords = np.ascontiguousarray(np.asarray(inputs["coords"], dtype=np.float32))
    shared = {
        "mass_w1": np.ascontiguousarray(np.asarray(inputs["mass_w1"], np.float32)),
        "mass_b1": np.ascontiguousarray(np.asarray(inputs["mass_b1"], np.float32)),
        "mass_w2": np.ascontiguousarray(np.asarray(inputs["mass_w2"], np.float32)),
        "mass_b2": np.ascontiguousarray(np.asarray(inputs["mass_b2"], np.float32)),
        "rff_W": np.ascontiguousarray(np.asarray(inputs["rff_W"], np.float32)),
        "rff_b": np.ascontiguousarray(np.asarray(inputs["rff_b"], np.float32)),
    }
    in_maps = [{"U": U[i], "coords": coords[i], **shared} for i in range(B)]
    res = run_bass_kernel_spmd(nc, in_maps, list(range(B)), trace=trace, tmpdir=tmpdir)
    out = np.stack([res.results[i]["out"].reshape(T, D, R_LR) for i in range(B)])
    return out.astype(np.float32), res


def kernel(**inputs) -> np.ndarray:
    out, _ = run(inputs, trace=False)
    return out


# revision 13
# speedup vs baseline: 1.3224x; 1.0290x over previous
"""GravityField Trainium2 kernel.

out = U * sqrt(1 + clip(0.1 * grav, -0.9, 5) + 1e-6)
where grav[t] = phi[t] . sum_t'(phi[t'] * mass[t']), phi = sqrt(2/R)*cos(coords@W+b),
mass = softplus(relu(coords@w1+b1)@w2+b2).

Sharding: pure data-parallel over B (8 batches -> 8 cores, no communication).
Each core processes coords [8192, 64] and U [8192, 512] (= 64*8 flattened).

v3 design (fp16 PE; DMA-floor ~85us at the ~420 GB/s shared R+W per-core rate):
- All matmuls fp16 (fp32 PE runs LOW_HIGH double-pass at 4 cyc/col). The RFF
  weights fold 1/2pi so the matmul yields angle-in-turns z; range reduction is
  two DVE ops: u = z + MAGIC, then fm = (u - MAGIC) - z = round(z) - z = -f via
  one scalar_tensor_tensor. phiT therefore holds -cos; the sign cancels because
  phi enters grav quadratically. Sin gets scale=6.28318 so |arg| <= 3.14159.
- Table churn: Relu+Sin co-reside in the trig act set; Sigmoid+Ln run once,
  batched over massT [16, 512]; one Sqrt set in phase 2.
- phi_sum: -mass bounced to DRAM f16, broadcast-read in 4 groups of 4 chunks,
  fused multiply+accumulate via scalar_tensor_tensor(accum_out).
- Phase 2 scale columns come straight from [128,1] matmuls with phiT blocks
  stationary (no K=1 transposes). t-rows interleave "(p j)" so coords DMAs are
  1KB/partition and U DMAs 8KB/partition contiguous.
- Queues: sync carries the 16 1MB U preloads from t=0 then the output writes;
  gpsimd carries weights, coords, massT row gathers, and the mass broadcasts.
"""

import sys

sys.path.insert(0, "/opt/trn_rl_repo")

import numpy as np
from contextlib import ExitStack

import concourse.bass as bass
import concourse.bacc as bacc
import concourse.mybir as mybir
from concourse import tile
from concourse.bass_utils import run_bass_kernel_spmd
from concourse.masks import make_identity

F32 = mybir.dt.float32
F16 = mybir.dt.float16
AF = mybir.ActivationFunctionType
ALU = mybir.AluOpType

B, T, D, R_LR, N_RFF = 8, 8192, 64, 8, 64
F = D * R_LR  # 512 floats of U per (b, t)
STRENGTH = 0.1
HALF_PI = 1.5707963267948966
INV_2PI = 0.15915494309189535
SIN_SCALE = 6.28318  # slightly under 2*pi: |SIN_SCALE * f| <= 3.14159 for f in [-.5,.5]
MAGIC = 12582912.0  # 1.5 * 2**23: fp32 add rounds to nearest integer
PHI_SUM_SCALE = STRENGTH * 2.0 / N_RFF
BIGC = 512
N_BIG = T // BIGC  # 16


def build_program():
    nc = bacc.Bacc("TRN2", target_bir_lowering=False, debug=False, num_devices=8)

    u_d = nc.dram_tensor("U", [T, F], F32, kind="ExternalInput")
    coords_d = nc.dram_tensor("coords", [T, D], F32, kind="ExternalInput")
    w1_d = nc.dram_tensor("mass_w1", [D, D], F32, kind="ExternalInput")
    b1_d = nc.dram_tensor("mass_b1", [D], F32, kind="ExternalInput")
    w2_d = nc.dram_tensor("mass_w2", [D, 1], F32, kind="ExternalInput")
    b2_d = nc.dram_tensor("mass_b2", [1], F32, kind="ExternalInput")
    rffw_d = nc.dram_tensor("rff_W", [D, N_RFF], F32, kind="ExternalInput")
    rffb_d = nc.dram_tensor("rff_b", [N_RFF], F32, kind="ExternalInput")
    out_d = nc.dram_tensor("out", [T, F], F32, kind="ExternalOutput")
    mscr_d = nc.dram_tensor("mscr", [1, T], F16)  # -mass bounce (f16)

    with tile.TileContext(nc) as tc, ExitStack() as ctx:
        const = ctx.enter_context(tc.tile_pool(name="const", bufs=1))

        u_tiles = [
            const.tile([128, 4 * F], F32, name=f"u{i}") for i in range(N_BIG)
        ]

        identity = const.tile([128, 128], F16)
        make_identity(nc, identity[:])

        # stationary operands need one producing engine -> bounce weights off DVE
        w_stage = const.tile([65, 128], F32)
        nc.scalar.dma_start(w_stage[0:64, 0:64], w1_d[:, :])
        nc.scalar.dma_start(w_stage[64:65, 0:64], b1_d[None, :])
        nc.scalar.dma_start(w_stage[0:64, 64:128], rffw_d[:, :])
        nc.scalar.dma_start(w_stage[64:65, 64:128], rffb_d[None, :])
        # angle in turns: z = coords @ (W/2pi) + (b + pi/2)/2pi; phi = sin(2pi z)
        nc.vector.tensor_scalar_mul(w_stage[0:64, 64:128], w_stage[0:64, 64:128], INV_2PI)
        nc.vector.tensor_scalar(
            w_stage[64:65, 64:128], w_stage[64:65, 64:128], HALF_PI, INV_2PI,
            op0=ALU.add, op1=ALU.mult,
        )
        w_comb = const.tile([65, 128], F16)
        nc.vector.tensor_copy(w_comb[:], w_stage[:])

        w2_stage = const.tile([D, 1], F32)
        nc.scalar.dma_start(w2_stage[:], w2_d[:, :])
        w2_sb = const.tile([D, 1], F16)
        nc.vector.tensor_copy(w2_sb[:], w2_stage[:])

        b2_stage = const.tile([N_BIG, 1], F32)
        nc.scalar.dma_start(b2_stage[:], b2_d[None, :].to_broadcast((N_BIG, 1)))
        b2_neg = const.tile([N_BIG, 1], F32)
        nc.vector.tensor_scalar_mul(b2_neg[:], b2_stage[:], -1.0)

        sqrt_bias = const.tile([128, 1], F32)
        nc.vector.memset(sqrt_bias[:], 1.000001)
        zero_bias = const.tile([N_RFF, 1], F32)
        nc.vector.memset(zero_bias[:], 0.0)

        massT = const.tile([N_BIG, BIGC], F32)   # mass pre-act, row per chunk
        phiT_all = const.tile([N_RFF, T], F16)   # -cos features, [R, T]
        partials = const.tile([N_RFF, 4], F32)
        phi_sum16 = const.tile([N_RFF, 1], F16)

        ct_pool = ctx.enter_context(tc.tile_pool(name="ct", bufs=4))
        rr_pool = ctx.enter_context(tc.tile_pool(name="rr", bufs=3))
        caug_pool = ctx.enter_context(tc.tile_pool(name="caug", bufs=3))
        # pre-set the ones row (bias input) on the rotating caug buffers
        for i in range(3):
            caug_init = caug_pool.tile([D + 1, BIGC], F16, tag="caug", name=f"ci{i}")
            nc.vector.memset(caug_init[D : D + 1, :], 1.0)
        h_pool = ctx.enter_context(tc.tile_pool(name="hT", bufs=2))
        f_pool = ctx.enter_context(tc.tile_pool(name="fm", bufs=3))
        mbc_pool = ctx.enter_context(tc.tile_pool(name="mbc", bufs=3))
        prod_pool = ctx.enter_context(tc.tile_pool(name="prod", bufs=2))
        sc_pool = ctx.enter_context(tc.tile_pool(name="sc", bufs=4))

        with (
            tc.tile_pool(name="ptp", bufs=2, space=bass.MemorySpace.PSUM) as tp_pool,
            tc.tile_pool(name="pbig", bufs=4, space=bass.MemorySpace.PSUM) as big_pool,
            tc.tile_pool(name="pmT", bufs=2, space=bass.MemorySpace.PSUM) as mT_pool,
        ):
            for c in range(N_BIG):
                tsl = slice(c * BIGC, (c + 1) * BIGC)
                nc.sync.dma_start(
                    u_tiles[c][:],
                    u_d[tsl, :].rearrange("(p j) f -> p (j f)", p=128),
                )
                # coords chunk, partition p <- rows 4p..4p+3 (1KB/partition)
                ct = ct_pool.tile([128, 4 * D], F32, tag="ct")
                nc.gpsimd.dma_start(
                    ct[:], coords_d[tsl, :].rearrange("(p j) d -> p (j d)", p=128)
                )

                ct16 = ct_pool.tile([128, 4 * D], F16, tag="ct16", bufs=3)
                nc.gpsimd.tensor_copy(ct16[:], ct[:])

                # transpose 4 blocks (fp16): phiT col j*128+q <-> t-row 4q+j
                tp = tp_pool.tile([D, BIGC], F16, tag="tp")
                for j in range(4):
                    nc.tensor.transpose(
                        tp[:, j * 128 : (j + 1) * 128],
                        ct16[:, j * D : (j + 1) * D],
                        identity[:],
                    )
                caug = caug_pool.tile([D + 1, BIGC], F16, tag="caug")
                nc.vector.tensor_copy(caug[0:D, :], tp[:])

                big = big_pool.tile([128, BIGC], F32, tag="big")
                nc.tensor.matmul(big[:], w_comb[:], caug[:], start=True, stop=True)

                # mass path: relu (trig act set; no table swap vs Sin)
                hT = h_pool.tile([D, BIGC], F16, tag="hT")
                nc.scalar.activation(hT[:], big[0:D, :], AF.Relu, bias=zero_bias[:])
                mT = mT_pool.tile([1, BIGC], F32, tag="mT")
                nc.tensor.matmul(mT[:], w2_sb[:], hT[:], start=True, stop=True)

                # range reduce: fm = round(z) - z = -f, exact, |fm| <= 0.5
                rru = rr_pool.tile([D, BIGC], F32, tag="rru")
                nc.vector.tensor_scalar_add(rru[:], big[D : 2 * D, :], MAGIC)
                fm = f_pool.tile([D, BIGC], F16, tag="fm")
                nc.vector.scalar_tensor_tensor(
                    fm[:], rru[:], MAGIC, big[D : 2 * D, :],
                    op0=ALU.subtract, op1=ALU.subtract,
                )
                # phiT = sin(2pi * -f) = -cos(angle); sign cancels in grav
                nc.scalar.activation(
                    phiT_all[:, tsl], fm[:], AF.Sin,
                    bias=zero_bias[:], scale=SIN_SCALE,
                )
                # engines can't write partition offset c (32-alignment rule);
                # copy to a partition-0 row then SBUF->SBUF DMA into massT[c]
                mrow = ct_pool.tile([1, BIGC], F32, tag="mrow", bufs=2)
                if c % 2 == 0:
                    nc.scalar.copy(mrow[:], mT[:])
                else:
                    nc.vector.tensor_copy(mrow[:], mT[:])
                nc.gpsimd.dma_start(massT[c : c + 1, :], mrow[:])

            # batched mass activation: -mass = ln(sigmoid(-(pre + b2)))
            msig = const.tile([N_BIG, BIGC], F32)
            nc.scalar.activation(msig[:], massT[:], AF.Sigmoid, bias=b2_neg[:], scale=-1.0)
            negm16 = const.tile([N_BIG, BIGC], F16)
            nc.scalar.activation(negm16[:], msig[:], AF.Ln)
            nc.scalar.dma_start(
                mscr_d[:, :].rearrange("a (c q) -> (a c) q", c=N_BIG), negm16[:]
            )
            for g in range(4):
                gsl = slice(g * 4 * BIGC, (g + 1) * 4 * BIGC)
                mbc = mbc_pool.tile([N_RFF, 4 * BIGC], F16, tag="mbc")
                eng = nc.gpsimd if g % 2 == 0 else nc.scalar
                eng.dma_start(
                    mbc[:], mscr_d[:, gsl].to_broadcast((N_RFF, 4 * BIGC))
                )
                prod = prod_pool.tile([N_RFF, 4 * BIGC], F16, tag="prod")
                nc.vector.scalar_tensor_tensor(
                    prod[:], phiT_all[:, gsl], 1.0, mbc[:],
                    op0=ALU.mult, op1=ALU.mult,
                    accum_out=partials[:, g : g + 1],
                )

            acc_raw = const.tile([N_RFF, 1], F32)
            nc.vector.reduce_sum(acc_raw[:], partials[:], axis=mybir.AxisListType.X)
            # acc = sum(phi*mass) (two sign flips cancel); phiT holds -cos, so
            # phi_sum16 = -PHI_SUM_SCALE * acc makes pg4 = +0.1*grav
            nc.scalar.mul(phi_sum16[:], acc_raw[:], -PHI_SUM_SCALE)

        with tc.tile_pool(name="pg", bufs=2, space=bass.MemorySpace.PSUM) as pg_pool:
            for g in range(N_BIG):
                tsl = slice(g * BIGC, (g + 1) * BIGC)
                # influence columns: pg4[q, j] = 0.1*grav(t = g*512 + 4q + j)
                pg4 = pg_pool.tile([128, 4], F32, tag="pg4")
                for j in range(4):
                    nc.tensor.matmul(
                        pg4[:, j : j + 1],
                        phiT_all[:, g * BIGC + j * 128 : g * BIGC + (j + 1) * 128],
                        phi_sum16[:],
                        start=True, stop=True,
                    )
                infl = sc_pool.tile([128, 4], F32, tag="infl")
                nc.vector.tensor_scalar(
                    infl[:], pg4[:], -0.9, 5.0, op0=ALU.max, op1=ALU.min
                )
                sc4 = sc_pool.tile([128, 4], F32, tag="sc4")
                nc.scalar.activation(sc4[:], infl[:], AF.Sqrt, bias=sqrt_bias[:])

                ut = u_tiles[g]
                for j in range(4):
                    usl = slice(j * F, (j + 1) * F)
                    if j % 2 == 0:
                        nc.vector.tensor_scalar_mul(ut[:, usl], ut[:, usl], sc4[:, j : j + 1])
                    else:
                        nc.scalar.mul(ut[:, usl], ut[:, usl], sc4[:, j : j + 1])
                nc.sync.dma_start(
                    out_d[tsl, :].rearrange("(p j) f -> p (j f)", p=128), ut[:]
                )

    nc.compile()
    return nc


_NC_CACHE = None


def _get_program():
    global _NC_CACHE
    if _NC_CACHE is None:
        _NC_CACHE = build_program()
    return _NC_CACHE


def run(inputs: dict, trace: bool = False, tmpdir=None):
    nc = _get_program()
    U = np.ascontiguousarray(np.asarray(inputs["U"], dtype=np.float32)).reshape(B, T, F)
    coords = np.ascontiguousarray(np.asarray(inputs["coords"], dtype=np.float32))
    shared = {
        "mass_w1": np.ascontiguousarray(np.asarray(inputs["mass_w1"], np.float32)),
        "mass_b1": np.ascontiguousarray(np.asarray(inputs["mass_b1"], np.float32)),
        "mass_w2": np.ascontiguousarray(np.asarray(inputs["mass_w2"], np.float32)),
        "mass_b2": np.ascontiguousarray(np.asarray(inputs["mass_b2"], np.float32)),
        "rff_W": np.ascontiguousarray(np.asarray(inputs["rff_W"], np.float32)),
        "rff_b": np.ascontiguousarray(np.asarray(inputs["rff_b"], np.float32)),
    }
    in_maps = [{"U": U[i], "coords": coords[i], **shared} for i in range(B)]
    res = run_bass_kernel_spmd(nc, in_maps, list(range(B)), trace=trace, tmpdir=tmpdir)
    out = np.stack([res.results[i]["out"].reshape(T, D, R_LR) for i in range(B)])
    return out.astype(np.float32), res


def kernel(**inputs) -> np.ndarray:
    out, _ = run(inputs, trace=False)
    return out
